# revision 42
# baseline (speedup 1.0000x reference)
"""Trainium2 Bass kernel for nn_AttentionBlock (B=8, C=512, H=W=64).

Sharding: data-parallel over batch. One batch element per NeuronCore,
8 cores, identical SPMD program, per-core inputs differ only in `x`.

Per-core pipeline (all activations [channels, n] with n = H*W = 4096):
  1. x arrives host-packed as fp8 DoubleRow slabs (2MB; the conv input
     directly -- zero device-side cast work) plus a lazy bf16 copy for
     the residual, off the startup critical path. Q/K/V weights arrive
     host-packed fp8 (scaled 16x to dodge fp8 subnormals). GroupNorm
     fp32 stats via bn_stats/bn_aggr (DVE) + ACT square-accumulate,
     group reduce/broadcast via tiny PE mask matmuls. The GN affine is
     folded into the conv weights: the fp8 weight slabs are rescaled
     in place by a16 = 16*a; biases ride separate tiny fp8 matmuls.
     Net conv scale 16*16*... = 4096 on bias path, 256 on activations,
     divided back out in the conv-output combine ops.
  2. Q/K/V 1x1 convs in fp8 DoubleRow (contraction 512 = 2 MMs of
     2x128), outputs cast to fp8 DR slabs.
  3. Attention over 8 query blocks of 512:
       QK: per key pair (16), S [P,2,512] over 4 DR MMs (2-bank PSUM
           tile); one ACT exp per pair (1024 elems) -> fp8 e slabs.
           Softmax denominator accumulated ON THE PE: one DR matmul
           per e-pair with a ones stationary vector into a dedicated
           [1,512] PSUM bank.
       r = exp(-ln(d)) on ACT. Every activation in the kernel (exp,
           ln, square, copy, identity) lives in ONE ACT table set so
           there are zero mid-kernel table reloads, and the ACT-based
           reciprocal is correctly modeled by the Tile scheduler (the
           DVE iterative-divide reciprocal is 6x under-modeled and
           made the scheduler park the PE on it).
       PV: out'[c,i] += V[j,c]^T E[j,i] in passes c0+c1, c2 (own
           block) and c3 (next block's QK window) so PSUM fits
           (sps 2x2 + ops 3 + dsum/rb 1 = 8 banks) and the PSUM-slot
           reuse gated on the r chain lands far from it; normalize
           into fp8 out slabs on DVE.
  4. proj conv (fp8 DR, host-packed 16x weights) + residual (resident
     bf16 x with the proj bias host-folded), interleaved into the next
     block's QK phase; fp32 out streamed to DRAM.
"""

import sys

import numpy as np

if "/opt/trn_rl_repo" not in sys.path:
    sys.path.insert(0, "/opt/trn_rl_repo")

B, C, HH, WW = 8, 512, 64, 64
N = HH * WW          # 4096
P = 128              # partitions
NPT = C // P         # 4 channel partition-tiles
NT = N // P          # 32 key tiles
NCH = N // 512       # 8 n-chunks / query blocks
GPP = 8              # groups per channel partition-tile (128/16)
CPG = 16             # channels per group
EPS = 1e-5
SCALE = float(1.0 / np.sqrt(C))
INV256 = 1.0 / 256.0
INV4096 = 1.0 / 4096.0

_CACHE = {}


def _build_program():
    import concourse.bacc as bacc
    import concourse.bass as bass
    import concourse.mybir as mybir
    from concourse import tile

    # Every activation this kernel uses (exp, ln, square, copy,
    # identity) lives in the single 'natural_log_exp_and_others' ACT
    # table, but the table-load placement pass picks the FIRST table
    # containing each function, ping-ponging between exp_and_others and
    # natural_log (1.3us reload each). Blank out every other table
    # (keeping list positions, which are the hardware set ids) so the
    # pass settles on the one shared table -> one load total.
    if not getattr(bacc, "_lnexp_tables_patch", False):
        from concourse import hw_specs

        _orig_gat = hw_specs.get_activation_tables

        def _only_lnexp(arch):
            return {k: (v if k == "natural_log_exp_and_others" else set())
                    for k, v in _orig_gat(arch).items()}

        bacc.get_activation_tables = _only_lnexp
        bacc._lnexp_tables_patch = True

    f32 = mybir.dt.float32
    bf16 = mybir.dt.bfloat16
    f8 = mybir.dt.float8e4
    DR = mybir.MatmulPerfMode.DoubleRow
    AF = mybir.ActivationFunctionType
    OP = mybir.AluOpType
    PSUM = bass.MemorySpace.PSUM

    nc = bacc.Bacc("TRN2", target_bir_lowering=False, debug=False,
                   enable_asserts=False)

    # host-packed inputs: one DMA per SBUF destination (per-DMA cost is
    # latency+queue-bandwidth bound; per-queue ~75GB/s)
    x8_d = nc.dram_tensor("x8", [P, 2, 2 * N], f8, kind="ExternalInput")
    xbf_d = nc.dram_tensor("xbf", [C, N], bf16, kind="ExternalInput")
    w8_d = nc.dram_tensor("w8", [P, 2, 6 * C], f8, kind="ExternalInput")
    wp8_d = nc.dram_tensor("wp8", [P, 2, 2 * C], f8, kind="ExternalInput")
    bcombo_d = nc.dram_tensor("bcombo", [P, 5 * NPT + GPP], f32,
                              kind="ExternalInput")
    bvrow_d = nc.dram_tensor("bvrow", [1, C], f32, kind="ExternalInput")
    gmaskT_d = nc.dram_tensor("gmaskT", [GPP, P], f32, kind="ExternalInput")
    out_d = nc.dram_tensor("out", [C, N], f32, kind="ExternalOutput")

    with tile.TileContext(nc) as tc:
        from contextlib import ExitStack

        with ExitStack() as root:
            # resident bf16 x for the final residual (lazy-loaded)
            xres_pool = root.enter_context(
                tc.tile_pool(name="xres", bufs=NPT))
            consts = root.enter_context(tc.tile_pool(name="consts", bufs=1))
            # out_sb tiles (normalized attention out, bf16)
            hpool = root.enter_context(tc.tile_pool(name="hpool", bufs=NPT))
            qkpool = root.enter_context(tc.tile_pool(name="qkpool",
                                                     bufs=2))
            vpool = root.enter_context(tc.tile_pool(name="vpool",
                                                    bufs=NT // 2))
            # fp8 x slabs + fp8 qkv weights live only until the convs
            # are done (released before the attention-phase pools open)
            qkv_scope = ExitStack()
            wqkv = qkv_scope.enter_context(
                tc.tile_pool(name="wqkv", bufs=1))
            x8t = wqkv.tile([P, 2, 2 * N], f8, tag="x8", name="x8t")
            w8t = wqkv.tile([P, 2, 6 * C], f8, tag="w8", name="w8t")
            wp8t = consts.tile([P, 2, 2 * C], f8, tag="wp8", name="wp8t")
            xres = [xres_pool.tile([P, N], bf16, tag="xr", name=f"xr{p}")
                    for p in range(NPT)]
            bcombo = consts.tile([P, 5 * NPT + GPP], f32, tag="bcombo",
                                 name="bcombo")
            bvrow = consts.tile([1, C], f32, tag="bvrow", name="bvrow")
            gmaskT = consts.tile([GPP, P], f32, tag="gmaskT", name="gmaskT")

            # DMA priority order (3 queues, ~70GB/s each); x8 chunk
            # (i, h) feeds GN channel tile p = 2h + i
            nc.sync.dma_start(gmaskT[:], gmaskT_d[:, :])
            nc.scalar.dma_start(bcombo[:], bcombo_d[:, :])
            nc.sync.dma_start(x8t[:, 0, 0:N], x8_d[:, 0, 0:N])
            nc.gpsimd.dma_start(x8t[:, 1, 0:N], x8_d[:, 1, 0:N])
            nc.scalar.dma_start(w8t[:], w8_d[:, :])
            nc.sync.dma_start(x8t[:, 0, N:2 * N], x8_d[:, 0, N:2 * N])
            nc.gpsimd.dma_start(x8t[:, 1, N:2 * N], x8_d[:, 1, N:2 * N])
            nc.scalar.dma_start(wp8t[:], wp8_d[:, :])
            nc.scalar.dma_start(bvrow[:], bvrow_d[:, :])
            # residual x loads ride sync after the startup-critical x8
            # chunks (needed only ~100us in, at the first proj)
            for p in range(NPT):
                nc.sync.dma_start(xres[p][:], xbf_d[p * P:(p + 1) * P, :])

            bsb = {nm: bcombo[:, i * NPT:(i + 1) * NPT]
                   for i, nm in enumerate(("bq", "bk", "bp", "gamma",
                                           "beta"))}
            gmask = bcombo[:, 5 * NPT:5 * NPT + GPP]

            def w8v(nm_i, h):
                # [P, 2, C] DR slab view for conv nm_i, channel half h
                if nm_i == 3:
                    return wp8t[:, :, h * C:(h + 1) * C]
                off = (nm_i * 2 + h) * C
                return w8t[:, :, off:off + C]

            def h8v(h):
                return x8t[:, :, h * N:(h + 1) * N]

            ones1b = consts.tile([1, P], bf16, tag="ones1b", name="ones1b")
            nc.vector.memset(ones1b[:], 1.0)
            ones1 = consts.tile([1, P], f32, tag="ones1", name="ones1")
            nc.vector.memset(ones1[:], 1.0)
            ones8 = consts.tile([P, 2, 16], f8, tag="ones8", name="ones8")
            nc.vector.memset(ones8[:], 1.0)
            eps_t = consts.tile([P, 1], f32, tag="eps", name="eps")
            nc.vector.memset(eps_t[:], EPS)
            # constant shift inside exp keeps E within fp8e4 range; it
            # cancels exactly in the softmax normalization
            shift_t = consts.tile([P, 1], f32, tag="shift", name="shift")
            nc.vector.memset(shift_t[:], -3.0)
            bvb = consts.tile([P, C], f32, tag="bvb", name="bvb")
            # DMA-free warm-matmul operand (memset, available ~6us)
            warmsrc = consts.tile([P, 512], bf16, tag="warmsrc",
                                  name="warmsrc")
            nc.vector.memset(warmsrc[:], 0.5)

            # fp8 DoubleRow slab layout note: dim 1 is the 2-way
            # contraction interleave; channel c = h*256 + i*128 + p for
            # half h, interleave i, partition p.
            q8 = [qkpool.tile([P, 2, N], f8, tag="q8", name=f"q8_{i}")
                  for i in range(2)]
            k8 = [qkpool.tile([P, 2, N], f8, tag="k8", name=f"k8_{i}")
                  for i in range(2)]
            v8 = [vpool.tile([P, 2, C], f8, tag="v8", name=f"v8_{i}")
                  for i in range(NT // 2)]

            # GroupNorm folded into the convs: h = x*a + b per channel;
            # w8 slabs get *= a16 in place, bias terms via tiny fp8 MMs.
            a16_ps = []
            b8s = []    # 16*b per channel (fp8, bias-MM moving operand)
            with tc.tile_pool(name="psA", bufs=2, space=PSUM) as psA, \
                 tc.tile_pool(name="smalls", bufs=8) as smalls:
                # keep the PE busy (and its HAM clock-gate warm) during
                # the GroupNorm stats latency with throwaway matmuls
                warm = psA.tile([P, 512], f32, tag="warm", bufs=1,
                                name="warm")

                def emit_warm(n):
                    for _ in range(n):
                        nc.tensor.matmul(warm[:], warmsrc[:, 0:P],
                                         warmsrc[:])

                emit_warm(72)
                # stats split: DVE bn_stats on 5 chunks, ACT square/sum
                # on 3 chunks. Phase 1 emits all tiles' heavy stats
                # back-to-back per engine (no cross-engine ping-pong);
                # phase 2 runs the small per-tile chains.
                hA = 5 * 512
                hB = N - hA
                stats_t = []
                for p in range(NPT):   # matches x8 chunk arrival order
                    h_i, i_i = p // 2, p % 2
                    x_p = x8t[:, i_i, h_i * N:(h_i + 1) * N]
                    # ACT squares+sums part B (scratch lands in the q8
                    # slab, overwritten later by the conv)
                    scr = q8[0][:, 0, 0:hB]
                    st4 = smalls.tile([P, 2], f32, tag="st4", name=f"st4{p}")
                    nc.scalar.activation(scr, x_p[:, hA:N], AF.Square,
                                         accum_out=st4[:, 1:2])
                    nc.scalar.activation(scr, x_p[:, hA:N], AF.Copy,
                                         accum_out=st4[:, 0:1])
                    bns = smalls.tile([P, 5 * 6], f32, tag="bns",
                                      name=f"bns{p}")
                    for s in range(5):
                        nc.vector.bn_stats(bns[:, s * 6:(s + 1) * 6],
                                           x_p[:, s * 512:(s + 1) * 512])
                    cst = smalls.tile([P, 2], f32, tag="cst", name=f"cst{p}")
                    nc.vector.bn_aggr(cst[:], bns[:])
                    stats = smalls.tile([P, 2], f32, tag="stats",
                                        name=f"stats{p}")
                    # s1 = mean_a*|A| + s1_b
                    nc.vector.scalar_tensor_tensor(
                        stats[:, 0:1], cst[:, 0:1], float(hA),
                        st4[:, 0:1], OP.mult, OP.add)
                    # m2_a = mean_a^2 + var_a ; s2 = m2_a*|A| + s2_b
                    m2a = smalls.tile([P, 1], f32, tag="m2a", name=f"m2a{p}")
                    nc.vector.scalar_tensor_tensor(
                        m2a[:], cst[:, 0:1], cst[:, 0:1], cst[:, 1:2],
                        OP.mult, OP.add)
                    nc.vector.scalar_tensor_tensor(
                        stats[:, 1:2], m2a[:], float(hA), st4[:, 1:2],
                        OP.mult, OP.add)
                    gst = psA.tile([GPP, 2], f32, tag="ps", name=f"gst{p}")
                    nc.tensor.matmul(gst[:], gmask[:], stats[:])
                    gsb = smalls.tile([GPP, 2], f32, tag="gsb",
                                      name=f"gsb{p}")
                    nc.vector.tensor_copy(gsb[:], gst[:])
                    stats_t.append(gsb)
                    emit_warm(8)
                # phase 2: per-tile group mean/var -> affine + fp8
                # weight rescale
                for p in range(NPT):
                    h_i, i_i = p // 2, p % 2
                    gsb = stats_t[p]
                    mu = smalls.tile([GPP, 2], f32, tag="mu", name=f"mu{p}")
                    nc.vector.tensor_scalar_mul(mu[:], gsb[:],
                                                1.0 / (CPG * N))
                    musq = smalls.tile([GPP, 1], f32, tag="musq",
                                       name=f"musq{p}")
                    nc.vector.tensor_tensor(musq[:], mu[:, 0:1], mu[:, 0:1],
                                            OP.mult)
                    var = smalls.tile([GPP, 1], f32, tag="var",
                                      name=f"var{p}")
                    nc.vector.tensor_tensor(var[:], mu[:, 1:2], musq[:],
                                            OP.subtract)
                    # rsig = exp(-0.5*ln(var+eps)): Ln and Exp share one
                    # activation table (natural_log_exp_and_others), so
                    # the whole kernel runs without mid-kernel ACT table
                    # reloads (Sqrt or DVE Reciprocal would force them)
                    lnv = smalls.tile([GPP, 1], f32, tag="lnv",
                                      name=f"lnv{p}")
                    nc.scalar.activation(lnv[:], var[:], AF.Ln,
                                         bias=eps_t[:GPP, 0:1])
                    rsqmu = smalls.tile([GPP, 2], f32, tag="rsqmu",
                                        name=f"rsqmu{p}")
                    nc.scalar.activation(rsqmu[:, 0:1], lnv[:], AF.Exp,
                                         scale=-0.5)
                    nc.vector.tensor_copy(rsqmu[:, 1:2], mu[:, 0:1])
                    bc = psA.tile([P, 2], f32, tag="ps", name=f"bc{p}")
                    nc.tensor.matmul(bc[:], gmaskT[:], rsqmu[:])
                    emit_warm(7)
                    # a16 = (16*gamma) * rsig  (gamma host-scaled by 16)
                    a16 = smalls.tile([P, 1], f32, tag="a", name=f"a{p}")
                    t_p = smalls.tile([P, 1], f32, tag="t", name=f"t{p}")
                    b_p = smalls.tile([P, 1], f32, tag="b", name=f"b{p}")
                    nc.vector.tensor_tensor(a16[:], bsb["gamma"][:, p:p + 1],
                                            bc[:, 0:1], OP.mult)
                    # t = mu * a = (mu/16) * a16 ; b = beta - t
                    nc.vector.tensor_scalar(t_p[:], bc[:, 1:2], 1.0 / 16,
                                            a16[:], OP.mult, OP.mult)
                    nc.vector.tensor_tensor(b_p[:], bsb["beta"][:, p:p + 1],
                                            t_p[:], OP.subtract)
                    # b8 = 16*b (fp8)
                    b8 = smalls.tile([P, 1], f8, tag="b8", name=f"b8{p}")
                    nc.vector.tensor_scalar_mul(b8[:], b_p[:], 16.0)
                    a16_ps.append((p, a16))
                    b8s.append((p, b8))
                    # rescale this channel tile's fp8 weight slices in
                    # place: w8 *= a16  (k on DVE, q on ACT, v on DVE)
                    nc.vector.tensor_scalar_mul(
                        w8v(1, h_i)[:, i_i, :], w8v(1, h_i)[:, i_i, :],
                        a16[:])
                    nc.scalar.activation(
                        w8v(0, h_i)[:, i_i, :], w8v(0, h_i)[:, i_i, :],
                        AF.Copy, scale=a16[:])
                    nc.vector.tensor_scalar_mul(
                        w8v(2, h_i)[:, i_i, :], w8v(2, h_i)[:, i_i, :],
                        a16[:])

                b8d = dict(b8s)
                # bias vectors: bias_q = Wq (a*b) + bq (column layout;
                # fp8 MMs carry 16w*16a*16b = 4096*w*a*b), and the v
                # bias as a partition-broadcast row
                biasqk = {}
                for nm_i, bias in ((0, "bq"), (1, "bk")):
                    bt = consts.tile([P, NPT], f32, tag=f"bias{nm_i}",
                                     name=f"bias{nm_i}")
                    for o in range(NPT):
                        bps = psA.tile([P, 1], f32, tag="ps",
                                       name=f"bps{nm_i}{o}")
                        k = 0
                        for h in range(2):
                            for i in range(2):
                                nc.tensor.matmul(
                                    bps[:],
                                    w8v(nm_i, h)[:, i, o * P:(o + 1) * P],
                                    b8d[2 * h + i][:], start=(k == 0),
                                    stop=(k == 3))
                                k += 1
                        nc.vector.tensor_scalar(bt[:, o:o + 1], bps[:],
                                                INV4096,
                                                bsb[bias][:, o:o + 1],
                                                OP.mult, OP.add)
                    biasqk[nm_i] = bt
                brow_ps = psA.tile([1, C], f32, tag="ps", name="brow_ps")
                k = 0
                for h in range(2):
                    for i in range(2):
                        nc.tensor.matmul(brow_ps[:], b8d[2 * h + i][:],
                                         w8v(2, h)[:, i, :], start=(k == 0),
                                         stop=(k == 3))
                        k += 1
                brow_sb = smalls.tile([1, C], f32, tag="brow", name="brow")
                nc.vector.scalar_tensor_tensor(brow_sb[:], brow_ps[:],
                                               INV4096, bvrow[:],
                                               OP.mult, OP.add)
                bvb_ps = psA.tile([P, C], f32, tag="ps", name="bvb_ps")
                nc.tensor.matmul(bvb_ps[:], ones1[:], brow_sb[:])
                nc.vector.tensor_copy(bvb[:], bvb_ps[:])

            # ---------------- QKV convs (fp8 DoubleRow) ----------------
            with tc.tile_pool(name="psC", bufs=8, space=PSUM) as psC:
                for nch in range(NCH):
                    sl = slice(nch * 512, (nch + 1) * 512)
                    for o in range(NPT):
                        kps = psC.tile([P, 512], f32, tag="c",
                                       name=f"kps{nch}_{o}")
                        for h in range(2):
                            nc.tensor.matmul(
                                kps[:], w8v(1, h)[:, :, o * P:(o + 1) * P],
                                h8v(h)[:, :, sl], start=(h == 0),
                                stop=(h == 1), perf_mode=DR)
                        nc.vector.tensor_scalar(k8[o // 2][:, o % 2, sl],
                                                kps[:], INV256,
                                                biasqk[1][:, o:o + 1],
                                                OP.mult, OP.add)
                    for o in range(NPT):
                        qps = psC.tile([P, 512], f32, tag="c",
                                       name=f"qps{nch}_{o}")
                        for h in range(2):
                            nc.tensor.matmul(
                                qps[:], w8v(0, h)[:, :, o * P:(o + 1) * P],
                                h8v(h)[:, :, sl], start=(h == 0),
                                stop=(h == 1), perf_mode=DR)
                        nc.scalar.activation(q8[o // 2][:, o % 2, sl],
                                             qps[:], AF.Identity,
                                             bias=biasqk[0][:, o:o + 1],
                                             scale=INV256)
                    for t in range(4):
                        nt = nch * 4 + t
                        vps = psC.tile([P, 512], f32, tag="c",
                                       name=f"vps{nt}")
                        for h in range(2):
                            nc.tensor.matmul(
                                vps[:], h8v(h)[:, :, nt * P:(nt + 1) * P],
                                w8v(2, h)[:, :, :], start=(h == 0),
                                stop=(h == 1), perf_mode=DR)
                        nc.vector.scalar_tensor_tensor(
                            v8[nt // 2][:, nt % 2, :], vps[:], INV256,
                            bvb[:], OP.mult, OP.add)

            # ------------- attention + interleaved proj -------------
            qkv_scope.close()  # release the fp8 x + qkv weight space
            out8 = [hpool.tile([P, 2, N], f8, tag="hb", name=f"osb{h}")
                    for h in range(2)]
            # PSUM: sps ring 2x2 banks + ops 3x1 + dsum/rb ring 1 = 8
            with tc.tile_pool(name="psS", bufs=2, space=PSUM) as psS, \
                 tc.tile_pool(name="psO", bufs=3, space=PSUM) as psO, \
                 tc.tile_pool(name="psD", bufs=1, space=PSUM) as psD, \
                 tc.tile_pool(name="att", bufs=4) as att, \
                 tc.tile_pool(name="epool", bufs=NT // 2 + 12) as epool, \
                 tc.tile_pool(name="fin", bufs=4) as fin:

                state = {}

                def emit_norm(ib, cs):
                    # out = out' * broadcast(1/d)  (DVE)
                    isl = slice(ib * 512, (ib + 1) * 512)
                    for c in cs:
                        nc.vector.scalar_tensor_tensor(
                            out8[c // 2][:, c % 2, isl],
                            state[ib]["ops"][c], 1.0,
                            state[ib]["rb"], OP.mult, OP.mult)

                def emit_proj_group(ib, o):
                    isl = slice(ib * 512, (ib + 1) * 512)
                    pps = psS.tile([P, 512], f32, tag="s",
                                   name=f"pps{ib}_{o}")
                    for h in range(2):
                        nc.tensor.matmul(
                            pps[:], w8v(3, h)[:, :, o * P:(o + 1) * P],
                            out8[h][:, :, isl], start=(h == 0),
                            stop=(h == 1), perf_mode=DR)
                    res = fin.tile([P, 512], f32, tag="res",
                                   name=f"res{ib}_{o}")
                    # xres arrives host-biased (x + bp); pps carries 16x
                    nc.vector.scalar_tensor_tensor(
                        res[:], pps[:], 1.0 / 16, xres[o][:, isl],
                        OP.mult, OP.add)
                    nc.sync.dma_start(out_d[o * P:(o + 1) * P, isl], res[:])

                def emit_qk(ib):
                    # block ib-1's tail (rb/norm/PV-c3/proj) interleaves
                    # into this QK phase: the reciprocal chain then has
                    # many matmuls of stream-distance before its first
                    # PE consumer, so the PE never drains on it.
                    isl = slice(ib * 512, (ib + 1) * 512)
                    prev = ib - 1 if ib > 0 else None
                    state[ib] = {}
                    es = []
                    proj_marks = {11: 0, 12: 1, 13: 2, 14: 3}
                    for t in range(NT // 2):
                        if prev is not None:
                            if t == 4:
                                emit_rb(prev)
                            elif t == 5:
                                emit_norm(prev, (0, 1, 2))
                            elif 6 <= t <= 9:
                                emit_pv_c3(prev, t - 6)
                            elif t == 10:
                                emit_norm(prev, (3,))
                            elif t in proj_marks:
                                emit_proj_group(prev, proj_marks[t])
                        sps = psS.tile([P, 2, 512], f32, tag="s",
                                       name=f"s{ib}_{t}")
                        for r in range(2):
                            j = 2 * t + r
                            for h in range(2):
                                nc.tensor.matmul(
                                    sps[:, r, :],
                                    k8[h][:, :, j * P:(j + 1) * P],
                                    q8[h][:, :, isl], start=(h == 0),
                                    stop=(h == 1), perf_mode=DR)
                        e_t = epool.tile([P, 2, 512], f8, tag="e",
                                         name=f"e{ib}_{t}")
                        es.append(e_t)
                        nc.scalar.activation(e_t[:], sps[:], AF.Exp,
                                             scale=SCALE,
                                             bias=shift_t[:, 0:1])
                        # softmax-denominator accumulation on the PE: one
                        # DR matmul per drained e-pair. The psD slot
                        # alternates rb(prev)/ds(ib), so ds is allocated
                        # after the t==4 rb(prev) mark, with a catch-up
                        # burst for pairs 0..3.
                        if t == 5:
                            state[ib]["dsum"] = psD.tile(
                                [1, 512], f32, tag="d", name=f"ds{ib}")
                            for u in range(4):
                                nc.tensor.matmul(state[ib]["dsum"][:],
                                                 ones8[:, :, 0:1],
                                                 es[u][:], start=(u == 0),
                                                 stop=False, perf_mode=DR)
                        elif t >= 6:
                            nc.tensor.matmul(state[ib]["dsum"][:],
                                             ones8[:, :, 0:1],
                                             es[t - 2][:], start=False,
                                             stop=False, perf_mode=DR)
                    t = NT // 2 - 2
                    nc.tensor.matmul(state[ib]["dsum"][:], ones8[:, :, 0:1],
                                     es[t][:], start=False, stop=False,
                                     perf_mode=DR)
                    state[ib]["es"] = es

                def emit_dtail(ib):
                    dsum = state[ib]["dsum"]
                    t = NT // 2 - 1
                    nc.tensor.matmul(dsum[:], ones8[:, :, 0:1],
                                     state[ib]["es"][t][:], start=False,
                                     stop=True, perf_mode=DR)
                    # r = exp(-ln(d)) on ACT: same shared table, ~1.2us,
                    # and accurately modeled by the scheduler (the DVE
                    # iterative-divide reciprocal is 3.3us but modeled as
                    # ~0.5us, which made the scheduler park the PE on it)
                    dln = att.tile([1, 512], f32, tag="dln", bufs=2,
                                   name=f"dln{ib}")
                    nc.scalar.activation(dln[:], dsum[:], AF.Ln)
                    r_sb = att.tile([1, 512], bf16, tag="r", bufs=2,
                                    name=f"r{ib}")
                    nc.scalar.activation(r_sb[:], dln[:], AF.Exp,
                                         scale=-1.0)
                    state[ib]["r"] = r_sb

                def emit_rb(ib):
                    # broadcast 1/d across partitions via ones matmul
                    rb_ps = psD.tile([P, 512], f32, tag="d",
                                     name=f"rb{ib}")
                    nc.tensor.matmul(rb_ps[:], ones1b[:], state[ib]["r"])
                    rb_sb = att.tile([P, 512], f32, tag="rb", bufs=2,
                                     name=f"rbs{ib}")
                    nc.vector.tensor_copy(rb_sb[:], rb_ps[:])
                    state[ib]["rb"] = rb_sb

                def emit_pv(ib):
                    # PV passes c0+c1 and c2 here; the c3 pass rides the
                    # next block's QK phase (after norm c0 frees its
                    # PSUM slot)
                    es = state[ib]["es"]
                    opsA = [psO.tile([P, 512], f32, tag="o",
                                     name=f"op{ib}_{c}") for c in (0, 1)]
                    for t in range(NT // 2):
                        if t == 3:
                            emit_dtail(ib)
                        for c in (0, 1):
                            nc.tensor.matmul(
                                opsA[c][:], v8[t][:, :, c * P:(c + 1) * P],
                                es[t][:], start=(t == 0),
                                stop=(t == NT // 2 - 1), perf_mode=DR)
                    opc2 = psO.tile([P, 512], f32, tag="o",
                                    name=f"op{ib}_2")
                    state[ib]["ops"] = opsA + [opc2]
                    for t in range(NT // 2):
                        nc.tensor.matmul(
                            opc2[:], v8[t][:, :, 2 * P:3 * P],
                            es[t][:], start=(t == 0),
                            stop=(t == NT // 2 - 1), perf_mode=DR)

                def emit_pv_c3(ib, chunk):
                    es = state[ib]["es"]
                    if chunk == 0:
                        opc3 = psO.tile([P, 512], f32, tag="o",
                                        name=f"op{ib}_3")
                        state[ib]["ops"] = state[ib]["ops"] + [opc3]
                    opc3 = state[ib]["ops"][3]
                    for t in range(chunk * 4, chunk * 4 + 4):
                        nc.tensor.matmul(
                            opc3[:], v8[t][:, :, 3 * P:4 * P],
                            es[t][:], start=(t == 0),
                            stop=(t == NT // 2 - 1), perf_mode=DR)

                for ib in range(NCH):
                    emit_qk(ib)
                    emit_pv(ib)
                last = NCH - 1
                # tail: the c3 pass borrows a free sps slot (the psO
                # ring slot would need norm c0, which needs the r
                # chain) so the PE chews it while r is computed
                opc3 = psS.tile([P, 512], f32, tag="s", name="op_t3")
                state[last]["ops"] = state[last]["ops"] + [opc3]
                for t in range(NT // 2):
                    nc.tensor.matmul(
                        opc3[:], v8[t][:, :, 3 * P:4 * P],
                        state[last]["es"][t][:], start=(t == 0),
                        stop=(t == NT // 2 - 1), perf_mode=DR)
                emit_rb(last)
                emit_norm(last, (0, 1, 2, 3))
                for o in range(NPT):
                    emit_proj_group(last, o)

    nc.compile()
    return nc


def _get_program():
    if "nc" not in _CACHE:
        _CACHE["nc"] = _build_program()
    return _CACHE["nc"]


def _make_in_maps(inputs):
    import ml_dtypes

    bf = ml_dtypes.bfloat16
    f8 = ml_dtypes.float8_e4m3
    f32 = np.float32

    def wpack8(w):
        # [C_in, C_out] -> fp8 DR slab block [P, 2, C] per half h:
        # [p, i, o] = 16*w[h*256 + i*128 + p, o]
        t = (np.asarray(w, dtype=f32).T * 16.0).reshape(2, 2, P, C)
        # t[h, i, p, o] -> per h: [p, i, o]
        return t.transpose(0, 2, 1, 3)   # [h, p, i, o]

    def colmaj(v):
        # [512] -> [128, 4] with out[p, t] = v[t*128 + p]
        return np.ascontiguousarray(
            np.asarray(v, dtype=f32).reshape(NPT, P).T)

    gm = np.zeros((P, GPP), f32)
    gm[np.arange(P), np.arange(P) // CPG] = 1.0
    bcombo = np.concatenate([
        colmaj(inputs["bq"]), colmaj(inputs["bk"]), colmaj(inputs["bp"]),
        # gamma scaled by 16 on host: a16 = 16*a rescales the fp8
        # weights in-device, staying out of fp8 subnormal range
        colmaj(np.asarray(inputs["gn_gamma"], dtype=f32) * 16.0),
        colmaj(inputs["gn_beta"]), gm,
    ], axis=1)
    # w8: [P, 2, 6C], conv order (q, k, v), per conv 2 halves;
    # wp8 separate (it must outlive the conv-phase SBUF scope)
    w8 = np.concatenate(
        [wpack8(inputs[nm])[h] for nm in ("wq", "wk", "wv")
         for h in range(2)], axis=2).astype(f8)
    wp8 = np.concatenate(
        [wpack8(inputs["wp"])[h] for h in range(2)], axis=2).astype(f8)
    common = {
        "w8": np.ascontiguousarray(w8),
        "wp8": np.ascontiguousarray(wp8),
        "bcombo": np.ascontiguousarray(bcombo),
        "bvrow": np.asarray(inputs["bv"], dtype=f32).reshape(1, C),
        "gmaskT": np.ascontiguousarray(gm.T),
    }
    x = np.asarray(inputs["x"], dtype=f32).reshape(B, C, N)
    # x8: [P, 2, 2N] fp8 DR slabs: [p, i, h*N + n] = x[h*256+i*128+p, n]
    x8 = x.reshape(B, 2, 2, P, N).transpose(0, 3, 2, 1, 4).reshape(
        B, P, 2, 2 * N).astype(f8)
    # residual with the proj bias pre-folded: out = proj + (x + bp)
    xbf = (x + np.asarray(inputs["bp"], dtype=f32)[None, :, None]
           ).astype(bf)
    return [dict(common, x8=np.ascontiguousarray(x8[i]),
                 xbf=np.ascontiguousarray(xbf[i])) for i in range(B)]


def run(inputs, trace=False):
    """Returns (output [B, C, H, W] fp32, BassKernelResults)."""
    from concourse import bass_utils

    nc = _get_program()
    in_maps = _make_in_maps(inputs)
    res = bass_utils.run_bass_kernel_spmd(nc, in_maps,
                                          core_ids=list(range(B)),
                                          trace=trace)
    out = np.stack([res.results[i]["out"] for i in range(B)], axis=0)
    return out.reshape(B, C, HH, WW).astype(np.float32), res


def kernel(**inputs):
    out, _ = run(inputs, trace=False)
    return out
